# revision 17
# baseline (speedup 1.0000x reference)
"""Causal self-attention for (2, 2048, 1024), 16 heads, on 8 trn2 cores.

Sharding: batch x head-group. Core c handles batch b = c // 4 and heads
[4*(c%4), 4*(c%4)+4). Each core computes q/k/v projections for its 4 heads,
runs causal attention per head-pair in transposed layout, applies its slice
of the output projection, and returns a [2048, 1024] bf16 partial. The host
sums the 4 partials per batch and adds the (folded) output bias.

v2 design notes (vs the 332us baseline):
- Head-pair scores matmuls are emitted back-to-back on opposite 64-row
  PE tiles (tile_position auto-derived from base partitions 0/64) so the
  hardware overlaps them.
- V is projected directly into [j, d] layout (stationary = xt j-tile,
  moving = Wv), eliminating all PE transposes.
- Attention runs per head-pair with an outer chunk-pair (i) loop so the
  full working set fits PSUM exactly: scores [128,2,512] (2 banks) +
  4 ctx accumulators [65,512] (4 banks) + 2 projection banks = 8.
- Projection matmul groups for the second head-pair (and V j-tiles, with
  a 2-tile lookahead) are interleaved between attention steps as PE
  filler, keeping the PE dense so the HAM clock gate stays at 2.4 GHz.
- Rowsum reciprocal via the single-instruction reciprocal_approx_fast
  (the baseline's 16x RECIPROCAL cost 53us of DVE time).
- The diagonal-block causal mask multiply runs on the (otherwise idle)
  GPSIMD engine.
- Inputs arrive pre-arranged so each weight/xt tensor is one contiguous
  DMA (the baseline spent ~27us of sync-engine time issuing 74 DMAs).
- Output partials are written bf16 (halves the output DMA).
- Biases: bq/bk are per-partition adds in the projection evacuation;
  bv and bo are folded host-side into a single output bias (exact:
  softmax rows sum to 1, so ctx(v + bv) = ctx(v) + bv).
"""

import sys

sys.path.insert(0, "/opt/trn_rl_repo")

import ml_dtypes
import numpy as np

import concourse.bass as bass
from concourse.bass import _add_dep_helper
import concourse.mybir as mybir
import concourse.tile as tile
from concourse.vector_clock import ScopedClock

B, S, H, NH, HD = 2, 2048, 1024, 16, 64
NCORES = 8
HPC = 4          # heads per core
CHUNK = 512      # i-chunk width (PSUM bank)
NIT = S // 128   # 16 i-tiles / j-tiles
NIC = S // CHUNK # 4 i-chunks
KT = H // 128    # 8 contraction tiles for projections
SCALE = 1.0 / np.sqrt(HD)

f32 = mybir.dt.float32
f32r = mybir.dt.float32r
bf16 = mybir.dt.bfloat16
EXP = mybir.ActivationFunctionType.Exp
MUL = mybir.AluOpType.mult
ADD = mybir.AluOpType.add


class _TC(tile.TileContext):
    """TileContext whose tail drain carries no sem waits: this walrus build
    rejects instructions with more than one sync-wait command, so the waits
    are emitted as individual wait_ge instructions instead."""

    def _drain_and_barrier(self, tick_clock, wait_clock):
        nc = self.nc
        carrier = nc.sync.nop()
        wait_clock.add_sem_waits(
            carrier.ins, ScopedClock({None: tick_clock.global_clock})
        )
        si = carrier.ins.sync_info
        waits = list(si.on_wait) if si and si.on_wait else []
        si.on_wait = []
        assert self.sems is not None
        id2handle = {h.num: h for h in self.sems.allocated().values()}
        for w in waits:
            nc.sync.wait_ge(id2handle[w.id], w.wait_value)
        nc.sync.drain()
        nc.all_engine_barrier()
        popped = nc._tile_sem_poison_stack.pop()
        assert popped is self._sem_poison
        nc.clear_and_free_semaphores(list(self.sems.allocated().values()))
        nc.all_engine_barrier()


_waitfix_ctr = [0]


def _split_multiwaits(nc):
    """Hoist all-but-one sync wait off every instruction into standalone
    single-wait EventSemaphore instructions (same engine, same position)."""
    for f in nc.m.functions:
        for bb in f.blocks:
            out = []
            changed = False
            for inst in bb.instructions:
                si = inst.sync_info
                waits = list(si.on_wait) if si and si.on_wait else []
                if len(waits) > 1:
                    changed = True
                    for w in waits[:-1]:
                        _waitfix_ctr[0] += 1
                        ev = mybir.InstEventSemaphore(
                            name=f"I-waitfix-{_waitfix_ctr[0]}",
                            engine=inst.engine,
                            ins=[],
                            outs=[],
                            sync_info=mybir.SyncInfo(on_wait=[w], on_update=[]),
                        )
                        nc.register_instruction(ev)
                        out.append(ev)
                    si.on_wait = waits[-1:]
                out.append(inst)
            if changed:
                bb.instructions = out


def _build_program():
    nc = bass.Bass("TRN2", target_bir_lowering=False, debug=False,
                   num_devices=NCORES)

    # Pre-arranged inputs: per-partition-contiguous so each is ONE fast DMA.
    xt = nc.dram_tensor("xt", [128, KT * S], bf16, kind="ExternalInput")
    wq = nc.dram_tensor("wq", [128, KT * 256], bf16, kind="ExternalInput")
    wk = nc.dram_tensor("wk", [128, KT * 256], bf16, kind="ExternalInput")
    wv = nc.dram_tensor("wv", [128, KT * 256], bf16, kind="ExternalInput")
    wo = nc.dram_tensor("wo", [128, 2 * H], bf16, kind="ExternalInput")
    bqkv = nc.dram_tensor("bqkv", [128, 4], f32, kind="ExternalInput")
    ones64 = nc.dram_tensor("ones64", [1, 64], bf16, kind="ExternalInput")
    mask2 = nc.dram_tensor("mask2", [128, 2 * 128], bf16, kind="ExternalInput")
    onesvj = nc.dram_tensor("onesvj", [128, NIT], bf16, kind="ExternalInput")
    outp = nc.dram_tensor("outp", [S, H], bf16, kind="ExternalOutput")

    last_pe = [None]

    def _mm(inst):
        if last_pe[0] is not None:
            _add_dep_helper(inst.ins, last_pe[0].ins, sync=False,
                            reason="pe emission order")
        last_pe[0] = inst
        return inst

    with _TC(nc) as tc:
        with (
            tc.tile_pool(name="const", bufs=1) as constp,
            tc.tile_pool(name="big", bufs=1) as bigp,
        ):
            # ---- constants (gpsimd SWDGE ring; sync ring kept for bulk) ---
            ones64_sb = constp.tile([1, 64], bf16)
            nc.gpsimd.dma_start(ones64_sb[:], ones64.ap())
            mask2_sb = constp.tile([128, 2, 128], bf16)
            nc.gpsimd.dma_start(mask2_sb[:], mask2.ap())
            bqkv_sb = constp.tile([128, 4], f32)
            nc.gpsimd.dma_start(bqkv_sb[:], bqkv.ap())
            # pre-warm the ACT exp table set (~2.7us) off the critical path
            warm_sb = constp.tile([1, 1], f32)
            nc.scalar.activation(warm_sb[:], bqkv_sb[0:1, 0:1], EXP,
                                 scale=1.0)

            # ---- bulk inputs on the sync HWDGE ring, priority order ------
            wk_sb = bigp.tile([128, KT, 256], bf16, name="wk_sb")
            nc.sync.dma_start(wk_sb[:], wk.ap())
            wq_sb = bigp.tile([128, KT, 256], bf16, name="wq_sb")
            nc.sync.dma_start(wq_sb[:], wq.ap())
            xt_sb = bigp.tile([128, KT, S], bf16, name="xt_sb")
            for t2 in range(4):  # 2 k-tiles per DMA: proj starts early
                nc.sync.dma_start(xt_sb[:, 2 * t2:2 * t2 + 2, :],
                                  xt.ap()[:, 2 * t2 * S:(2 * t2 + 2) * S])
            wv_sb = bigp.tile([128, KT, 256], bf16, name="wv_sb")
            nc.sync.dma_start(wv_sb[:], wv.ap())
            wo_sb = bigp.tile([128, 2, H], bf16, name="wo_sb")
            nc.sync.dma_start(wo_sb[:], wo.ap())

            # ---- persistent SBUF state -----------------------------------
            qt_sb = bigp.tile([128, 2, S], bf16, name="qt_sb")
            kt_sb = bigp.tile([128, 2, S], bf16, name="kt_sb")
            # v in [j, head-strided d | ones] layout: head h at cols
            # 65h..65h+63, ones column at 65h+64
            vj_sb = bigp.tile([128, NIT, HPC * (HD + 1)], bf16, name="vj_sb")
            for h in range(HPC):
                nc.gpsimd.dma_start(
                    vj_sb[:, :, h * (HD + 1) + HD:h * (HD + 1) + HD + 1],
                    onesvj.ap(),
                )
            ctxT2_sb = bigp.tile([128, 2, S], bf16, name="ctxT2_sb")

            # ---- projection emitters (called as attention filler) --------
            def emit_kq_group(projp, name, w_sb, dst, plane, ic):
                """One [128,512] output chunk of a k/q projection plane."""
                ps = projp.tile([128, CHUNK], f32, tag="pj",
                                name=f"pj_{name}{plane}_{ic}")
                for t in range(KT):
                    _mm(nc.tensor.matmul(
                        ps[:],
                        w_sb[:, t, plane * 128:(plane + 1) * 128],
                        xt_sb[:, t, ic * CHUNK:(ic + 1) * CHUNK],
                        start=(t == 0),
                        stop=(t == KT - 1),
                    ))
                boff = {"q": 0, "k": 2}[name]
                nc.vector.tensor_scalar(
                    out=dst[:, plane, ic * CHUNK:(ic + 1) * CHUNK],
                    in0=ps[:],
                    scalar1=bqkv_sb[:, boff + plane:boff + plane + 1],
                    scalar2=None,
                    op0=ADD,
                )

            def emit_v_group(projp, jt):
                """v[j, d] for all 4 heads, one j-tile (stationary = xt)."""
                ps = projp.tile([128, CHUNK], f32, tag="pj", name=f"pv_{jt}")
                for t in range(KT):
                    _mm(nc.tensor.matmul(
                        ps[:, 0:256],
                        xt_sb[:, t, jt * 128:(jt + 1) * 128],
                        wv_sb[:, t, :],
                        start=(t == 0),
                        stop=(t == KT - 1),
                    ))
                # strided evac: head h -> cols 65h..65h+64
                nc.vector.tensor_copy(
                    vj_sb[:, jt, :].rearrange("p (h x) -> p h x", x=HD + 1)
                    [:, :, 0:HD],
                    ps[:, 0:256].rearrange("p (h d) -> p h d", d=HD),
                )

            # ---- attention for one head pair -----------------------------
            def attention_pair(p, scp, ctxp, rsp, ptp, filler):
                """Heads (2p, 2p+1). filler: iterator of 0-arg callables
                emitted one per attention step (PE work during exp)."""
                for icp in range(2):
                    ics = (2 * icp, 2 * icp + 1)
                    ctx_t = {}
                    for hp in range(2):
                        for ic in ics:
                            ctx_t[(hp, ic)] = ctxp.tile(
                                [HD + 1, CHUNK], f32, tag=f"ctx{hp}{ic % 2}",
                                name=f"ctx_{p}_{hp}_{ic}_{icp}")
                    for jt in range(8 * icp + 8):
                        for ic in ics:
                            if 128 * jt >= CHUNK * (ic + 1):
                                continue
                            off = max(0, 128 * jt - CHUNK * ic)
                            # filler first: runs on PE during the exp below
                            for fn in filler[:1]:
                                fn()
                            del filler[:1]
                            sc = scp.tile([128, 2, CHUNK], f32, tag="sc")
                            for hp in range(2):
                                _mm(nc.tensor.matmul(
                                    sc[:, hp, off:CHUNK],
                                    kt_sb[hp * 64:hp * 64 + 64, p,
                                          jt * 128:(jt + 1) * 128],
                                    qt_sb[hp * 64:hp * 64 + 64, p,
                                          ic * CHUNK + off:(ic + 1) * CHUNK],
                                    start=True,
                                    stop=True,
                                ))
                            pt = ptp.tile([128, 2, CHUNK], bf16, tag="pt")
                            nc.scalar.activation(
                                pt[:, :, off:CHUNK],
                                sc[:, :, off:CHUNK],
                                EXP,
                                scale=float(SCALE),
                            )
                            if jt // 4 == ic:  # diagonal block
                                nc.gpsimd.tensor_tensor(
                                    out=pt[:, :, off:off + 128],
                                    in0=pt[:, :, off:off + 128],
                                    in1=mask2_sb[:],
                                    op=MUL,
                                )
                            for hp in range(2):
                                h = 2 * p + hp
                                _mm(nc.tensor.matmul(
                                    ctx_t[(hp, ic)][:, off:CHUNK],
                                    vj_sb[:, jt,
                                          h * (HD + 1):(h + 1) * (HD + 1)],
                                    pt[:, hp, off:CHUNK],
                                    start=(jt == 0),
                                    stop=(jt == 4 * ic + 3),
                                ))
                    # normalize + store this chunk-pair
                    for hp in range(2):
                        for ic in ics:
                            ct = ctx_t[(hp, ic)]
                            rsb = rsp.tile([1, CHUNK], bf16, tag="rsb")
                            with nc.allow_low_precision(
                                    reason="rowsum reciprocal to bf16"):
                                nc.vector.reciprocal(rsb[:], ct[HD:HD + 1, :])
                            bc_t = scp.tile([128, 2, CHUNK], f32, tag="sc")
                            bc = bc_t[0:HD, 0, :]
                            _mm(nc.tensor.matmul(
                                bc,
                                ones64_sb[:],
                                rsb[:],
                                start=True,
                                stop=True,
                            ))
                            bcs = rsp.tile([HD, CHUNK], f32, tag="bcs")
                            nc.vector.tensor_copy(bcs[:], bc)
                            nc.vector.tensor_tensor(
                                out=ctxT2_sb[hp * 64:hp * 64 + 64, p,
                                             ic * CHUNK:(ic + 1) * CHUNK],
                                in0=bcs[:],
                                in1=ct[0:HD, :],
                                op=MUL,
                            )

            # ---- phase structure -----------------------------------------
            with (
                tc.tile_pool(name="proj", bufs=2, space="PSUM") as projp,
                tc.tile_pool(name="sc0", bufs=1, space="PSUM") as scp0,
                tc.tile_pool(name="ctx", bufs=1, space="PSUM") as ctxp,
                tc.tile_pool(name="rs", bufs=4) as rsp,
                tc.tile_pool(name="pt", bufs=3) as ptp,
            ):
                # upfront: k/q plane 0, then v j-tiles 0-1 (ctx lookahead)
                for ic in range(NIC):
                    emit_kq_group(projp, "k", wk_sb, kt_sb, 0, ic)
                for ic in range(NIC):
                    emit_kq_group(projp, "q", wq_sb, qt_sb, 0, ic)
                emit_v_group(projp, 0)
                emit_v_group(projp, 1)

                # filler for pair 0: v j-tiles (2 ahead), then k/q plane 1
                filler = []
                for jt in range(2, NIT):
                    filler.append(lambda jt=jt: emit_v_group(projp, jt))
                for ic in range(NIC):
                    filler.append(lambda ic=ic: emit_kq_group(
                        projp, "k", wk_sb, kt_sb, 1, ic))
                for ic in range(NIC):
                    filler.append(lambda ic=ic: emit_kq_group(
                        projp, "q", wq_sb, qt_sb, 1, ic))

                attention_pair(0, scp0, ctxp, rsp, ptp, filler)
                # any leftover filler (shouldn't be: 22 groups vs 40 steps)
                for fn in filler:
                    fn()
                filler = []

            with (
                tc.tile_pool(name="sc1", bufs=2, space="PSUM") as scp1,
                tc.tile_pool(name="ctx1", bufs=1, space="PSUM") as ctxp1,
                tc.tile_pool(name="rs1", bufs=4) as rsp1,
                tc.tile_pool(name="pt1", bufs=3) as ptp1,
            ):
                attention_pair(1, scp1, ctxp1, rsp1, ptp1, [])

            # ---- output projection ---------------------------------------
            with (
                tc.tile_pool(name="om", bufs=4, space="PSUM") as omp,
                tc.tile_pool(name="osb", bufs=4) as osbp,
            ):
                for it in range(NIT):
                    pso = [omp.tile([128, CHUNK], f32, tag="om",
                                    name=f"om_{it}_{nck}")
                           for nck in range(2)]
                    for p in range(2):
                        for nck in range(2):
                            _mm(nc.tensor.matmul(
                                pso[nck][:],
                                ctxT2_sb[:, p, it * 128:(it + 1) * 128],
                                wo_sb[:, p, nck * CHUNK:(nck + 1) * CHUNK],
                                start=(p == 0),
                                stop=(p == 1),
                            ))
                    osb = osbp.tile([128, 2, CHUNK], bf16, tag="osb")
                    for nck in range(2):
                        nc.vector.tensor_copy(osb[:, nck, :], pso[nck][:])
                    nc.sync.dma_start(
                        outp.ap()[it * 128:(it + 1) * 128, :],
                        osb[:],
                    )

    _split_multiwaits(nc)
    return nc


_nc_cache = None


def _get_program():
    global _nc_cache
    if _nc_cache is None:
        _nc_cache = _build_program()
    return _nc_cache


def kernel(hidden_states, Wq, bq, Wk, bk, Wv, bv, Wo, bo):
    from concourse.bass_utils import run_bass_kernel_spmd

    hidden_states = np.asarray(hidden_states, dtype=np.float32)
    Wq, bq = np.asarray(Wq, np.float32), np.asarray(bq, np.float32)
    Wk, bk = np.asarray(Wk, np.float32), np.asarray(bk, np.float32)
    Wv, bv = np.asarray(Wv, np.float32), np.asarray(bv, np.float32)
    Wo, bo = np.asarray(Wo, np.float32), np.asarray(bo, np.float32)

    ones64 = np.ones((1, 64), ml_dtypes.bfloat16)
    # mask[j_local, plane, i_local] = 1 where i >= j inside a diagonal block
    m = np.tril(np.ones((128, 128), np.float32)).T
    mask2 = np.ascontiguousarray(
        np.stack([m, m], axis=1).reshape(128, 256)).astype(ml_dtypes.bfloat16)
    onesvj = np.ones((128, NIT), ml_dtypes.bfloat16)

    def ktile_major(a, kt):  # [kt*128, F] -> [128, kt*F] partition-contig
        f = a.shape[1]
        return np.ascontiguousarray(
            a.reshape(kt, 128, f).transpose(1, 0, 2).reshape(128, kt * f)
        ).astype(ml_dtypes.bfloat16)

    in_maps = []
    for c in range(NCORES):
        b = c // (NCORES // B)
        hg = c % (NCORES // B)
        hsel = slice(hg * HPC * HD, (hg + 1) * HPC * HD)
        xt_n = ktile_major(np.ascontiguousarray(hidden_states[b].T), KT)
        bq_c = bq[hsel].reshape(2, 128).T
        bk_c = bk[hsel].reshape(2, 128).T
        bqkv_c = np.concatenate([bq_c, bk_c], axis=1).astype(np.float32)
        in_maps.append({
            "xt": xt_n,
            "wq": ktile_major(np.ascontiguousarray(Wq[:, hsel]), KT),
            "wk": ktile_major(np.ascontiguousarray(Wk[:, hsel]), KT),
            "wv": ktile_major(np.ascontiguousarray(Wv[:, hsel]), KT),
            "wo": ktile_major(np.ascontiguousarray(Wo[hsel, :]), 2),
            "bqkv": np.ascontiguousarray(bqkv_c),
            "ones64": ones64,
            "mask2": mask2,
            "onesvj": onesvj,
        })

    res = run_bass_kernel_spmd(_get_program(), in_maps, list(range(NCORES)))
    out = np.zeros((B, S, H), np.float32)
    for c in range(NCORES):
        out[c // (NCORES // B)] += np.asarray(
            res.results[c]["outp"]).astype(np.float32)
    out += (bo + bv @ Wo)[None, None, :]
    return out


# revision 23
# speedup vs baseline: 1.5835x; 1.5835x over previous
"""Causal self-attention for (2, 2048, 1024), 16 heads, on 8 trn2 cores.

Sharding: batch x head-group. Core c handles batch b = c // 4 and heads
[4*(c%4), 4*(c%4)+4). Each core computes q/k/v projections for its 4 heads,
runs causal attention per head-pair in transposed layout, applies its slice
of the output projection, and returns a [2048, 1024] bf16 partial. The host
sums the 4 partials per batch and adds the (folded) output bias.

v2 design notes (vs the 332us baseline):
- Head-pair scores matmuls are emitted back-to-back on opposite 64-row
  PE tiles (tile_position auto-derived from base partitions 0/64) so the
  hardware overlaps them.
- V is projected directly into [j, d] layout (stationary = xt j-tile,
  moving = Wv), eliminating all PE transposes.
- Attention runs per head-pair with an outer chunk-pair (i) loop so the
  full working set fits PSUM exactly: scores [128,2,512] (2 banks) +
  4 ctx accumulators [65,512] (4 banks) + 2 projection banks = 8.
- Projection matmul groups for the second head-pair (and V j-tiles, with
  a 2-tile lookahead) are interleaved between attention steps as PE
  filler, keeping the PE dense so the HAM clock gate stays at 2.4 GHz.
- Rowsum reciprocal via the single-instruction reciprocal_approx_fast
  (the baseline's 16x RECIPROCAL cost 53us of DVE time).
- The diagonal-block causal mask multiply runs on the (otherwise idle)
  GPSIMD engine.
- Inputs arrive pre-arranged so each weight/xt tensor is one contiguous
  DMA (the baseline spent ~27us of sync-engine time issuing 74 DMAs).
- Output partials are written bf16 (halves the output DMA).
- Biases: bq/bk are per-partition adds in the projection evacuation;
  bv and bo are folded host-side into a single output bias (exact:
  softmax rows sum to 1, so ctx(v + bv) = ctx(v) + bv).
"""

import sys

sys.path.insert(0, "/opt/trn_rl_repo")

import ml_dtypes
import numpy as np

import concourse.bass as bass
from concourse.bass import _add_dep_helper
import concourse.mybir as mybir
import concourse.tile as tile
from concourse.vector_clock import ScopedClock

B, S, H, NH, HD = 2, 2048, 1024, 16, 64
NCORES = 8
HPC = 4          # heads per core
CHUNK = 512      # i-chunk width (PSUM bank)
NIT = S // 128   # 16 i-tiles / j-tiles
NIC = S // CHUNK # 4 i-chunks
KT = H // 128    # 8 contraction tiles for projections
SCALE = 1.0 / np.sqrt(HD)

f32 = mybir.dt.float32
f32r = mybir.dt.float32r
bf16 = mybir.dt.bfloat16
EXP = mybir.ActivationFunctionType.Exp
MUL = mybir.AluOpType.mult
ADD = mybir.AluOpType.add


class _TC(tile.TileContext):
    """TileContext whose tail drain carries no sem waits: this walrus build
    rejects instructions with more than one sync-wait command, so the waits
    are emitted as individual wait_ge instructions instead."""

    def _drain_and_barrier(self, tick_clock, wait_clock):
        nc = self.nc
        carrier = nc.sync.nop()
        wait_clock.add_sem_waits(
            carrier.ins, ScopedClock({None: tick_clock.global_clock})
        )
        si = carrier.ins.sync_info
        waits = list(si.on_wait) if si and si.on_wait else []
        si.on_wait = []
        assert self.sems is not None
        id2handle = {h.num: h for h in self.sems.allocated().values()}
        for w in waits:
            nc.sync.wait_ge(id2handle[w.id], w.wait_value)
        nc.sync.drain()
        nc.all_engine_barrier()
        popped = nc._tile_sem_poison_stack.pop()
        assert popped is self._sem_poison
        nc.clear_and_free_semaphores(list(self.sems.allocated().values()))
        nc.all_engine_barrier()


_waitfix_ctr = [0]


def _split_multiwaits(nc):
    """Hoist all-but-one sync wait off every instruction into standalone
    single-wait EventSemaphore instructions (same engine, same position)."""
    for f in nc.m.functions:
        for bb in f.blocks:
            out = []
            changed = False
            for inst in bb.instructions:
                si = inst.sync_info
                waits = list(si.on_wait) if si and si.on_wait else []
                if len(waits) > 1:
                    changed = True
                    for w in waits[:-1]:
                        _waitfix_ctr[0] += 1
                        ev = mybir.InstEventSemaphore(
                            name=f"I-waitfix-{_waitfix_ctr[0]}",
                            engine=inst.engine,
                            ins=[],
                            outs=[],
                            sync_info=mybir.SyncInfo(on_wait=[w], on_update=[]),
                        )
                        nc.register_instruction(ev)
                        out.append(ev)
                    si.on_wait = waits[-1:]
                out.append(inst)
            if changed:
                bb.instructions = out


def _build_program():
    nc = bass.Bass("TRN2", target_bir_lowering=False, debug=False,
                   num_devices=NCORES)

    # Pre-arranged inputs: per-partition-contiguous so each is ONE fast DMA.
    xt = nc.dram_tensor("xt", [128, KT * S], bf16, kind="ExternalInput")
    wq = nc.dram_tensor("wq", [128, KT * 256], bf16, kind="ExternalInput")
    wk = nc.dram_tensor("wk", [128, KT * 256], bf16, kind="ExternalInput")
    wv = nc.dram_tensor("wv", [128, KT * 256], bf16, kind="ExternalInput")
    wo = nc.dram_tensor("wo", [128, 2 * H], bf16, kind="ExternalInput")
    bqkv = nc.dram_tensor("bqkv", [128, 4], f32, kind="ExternalInput")
    ones64 = nc.dram_tensor("ones64", [1, 64], bf16, kind="ExternalInput")
    mask2 = nc.dram_tensor("mask2", [128, 2 * 128], bf16, kind="ExternalInput")
    onesvj = nc.dram_tensor("onesvj", [128, NIT], bf16, kind="ExternalInput")
    outp = nc.dram_tensor("outp", [S, H], bf16, kind="ExternalOutput")

    last_pe = [None]

    def _mm(inst):
        if last_pe[0] is not None:
            _add_dep_helper(inst.ins, last_pe[0].ins, sync=False,
                            reason="pe emission order")
        last_pe[0] = inst
        return inst

    with _TC(nc) as tc:
        with (
            tc.tile_pool(name="const", bufs=1) as constp,
            tc.tile_pool(name="big", bufs=1) as bigp,
        ):
            # ---- constants (gpsimd SWDGE ring; sync ring kept for bulk) ---
            ones64_sb = constp.tile([1, 64], bf16)
            nc.gpsimd.dma_start(ones64_sb[:], ones64.ap())
            mask2_sb = constp.tile([128, 2, 128], bf16)
            nc.gpsimd.dma_start(mask2_sb[:], mask2.ap())
            bqkv_sb = constp.tile([128, 4], f32)
            nc.gpsimd.dma_start(bqkv_sb[:], bqkv.ap())
            # pre-warm the ACT exp table set (~2.7us) off the critical path
            warm_sb = constp.tile([1, 1], f32)
            nc.scalar.activation(warm_sb[:], bqkv_sb[0:1, 0:1], EXP,
                                 scale=1.0)

            # ---- bulk inputs on the sync HWDGE ring, priority order ------
            wk_sb = bigp.tile([128, KT, 256], bf16, name="wk_sb")
            nc.sync.dma_start(wk_sb[:], wk.ap())
            wq_sb = bigp.tile([128, KT, 256], bf16, name="wq_sb")
            nc.sync.dma_start(wq_sb[:], wq.ap())
            xt_sb = bigp.tile([128, KT, S], bf16, name="xt_sb")
            for t2 in range(4):  # 2 k-tiles per DMA: proj starts early
                nc.sync.dma_start(xt_sb[:, 2 * t2:2 * t2 + 2, :],
                                  xt.ap()[:, 2 * t2 * S:(2 * t2 + 2) * S])
            wv_sb = bigp.tile([128, KT, 256], bf16, name="wv_sb")
            nc.sync.dma_start(wv_sb[:], wv.ap())
            wo_sb = bigp.tile([128, 2, H], bf16, name="wo_sb")
            nc.sync.dma_start(wo_sb[:], wo.ap())

            # ---- persistent SBUF state -----------------------------------
            qt_sb = bigp.tile([128, 2, S], bf16, name="qt_sb")
            kt_sb = bigp.tile([128, 2, S], bf16, name="kt_sb")
            # v in [j, head-strided d | ones] layout: head h at cols
            # 65h..65h+63, ones column at 65h+64
            vj_sb = bigp.tile([128, NIT, HPC * (HD + 1)], bf16, name="vj_sb")
            for h in range(HPC):
                nc.gpsimd.dma_start(
                    vj_sb[:, :, h * (HD + 1) + HD:h * (HD + 1) + HD + 1],
                    onesvj.ap(),
                )
            ctxT2_sb = bigp.tile([128, 2, S], bf16, name="ctxT2_sb")

            # ---- projection emitters (called as attention filler) --------
            def emit_kq_group(pool, name, w_sb, dst, plane, ic):
                """One [128,512] output chunk of a k/q projection plane."""
                ps_t = pool.tile([128, 2, CHUNK], f32, tag="sc",
                                 name=f"pj_{name}{plane}_{ic}")
                ps = ps_t[:, 0, :]
                for t in range(KT):
                    _mm(nc.tensor.matmul(
                        ps,
                        w_sb[:, t, plane * 128:(plane + 1) * 128],
                        xt_sb[:, t, ic * CHUNK:(ic + 1) * CHUNK],
                        start=(t == 0),
                        stop=(t == KT - 1),
                    ))
                boff = {"q": 0, "k": 2}[name]
                nc.vector.tensor_scalar(
                    out=dst[:, plane, ic * CHUNK:(ic + 1) * CHUNK],
                    in0=ps,
                    scalar1=bqkv_sb[:, boff + plane:boff + plane + 1],
                    scalar2=None,
                    op0=ADD,
                )

            def emit_v_group(pool, jt):
                """v[j, d] for all 4 heads, one j-tile (stationary = xt)."""
                ps_t = pool.tile([128, 2, CHUNK], f32, tag="sc",
                                 name=f"pv_{jt}")
                ps = ps_t[:, 0, 0:256]
                for t in range(KT):
                    _mm(nc.tensor.matmul(
                        ps,
                        xt_sb[:, t, jt * 128:(jt + 1) * 128],
                        wv_sb[:, t, :],
                        start=(t == 0),
                        stop=(t == KT - 1),
                    ))
                # strided evac: head h -> cols 65h..65h+64
                nc.vector.tensor_copy(
                    vj_sb[:, jt, :].rearrange("p (h x) -> p h x", x=HD + 1)
                    [:, :, 0:HD],
                    ps.rearrange("p (h d) -> p h d", d=HD),
                )

            # ---- attention machinery -------------------------------------
            # All PSUM flows through two pools: scp ("sc", 2 bufs of
            # [128,2,512] = 4 banks) serving scores, projection groups,
            # rsinv broadcasts and outproj; ctxp (4 tags x 1 buf = 4 banks)
            # holding the per-(head, chunk) ctx accumulators.
            LAG = 4  # ctx matmuls trail scores/exp by this many steps

            def normalize_phase2(p, scp, rsp, hp, ic, ct, rsb):
                """Broadcast 1/rowsum and scale+store the ctx chunk.
                Emitted a few steps AFTER the reciprocal so the PE-queue
                bc matmul never head-of-line blocks on the DVE."""
                bc_t = scp.tile([128, 2, CHUNK], f32, tag="sc",
                                name=f"bc_{p}_{hp}_{ic}")
                bc = bc_t[0:HD, 0, :]
                _mm(nc.tensor.matmul(
                    bc, ones64_sb[:], rsb[:], start=True, stop=True))
                bcs = rsp.tile([HD, CHUNK], f32, tag="bcs")
                nc.vector.tensor_copy(bcs[:], bc)
                nc.vector.tensor_tensor(
                    out=ctxT2_sb[hp * 64:hp * 64 + 64, p,
                                 ic * CHUNK:(ic + 1) * CHUNK],
                    in0=bcs[:],
                    in1=ct[0:HD, :],
                    op=MUL,
                )

            def attention_pair(p, scp, ctxp, rsp, ptp, side, post):
                """Heads (2p, 2p+1). side: ordered 0-arg callables (proj /
                outproj groups) emitted one per step as PE filler. post:
                deferred normalize_phase2 closures (consumed with priority,
                refilled by this pair for the next phase)."""
                for icp in range(2):
                    ics = (2 * icp, 2 * icp + 1)
                    ctx_t = {}
                    for hp in range(2):
                        for ic in ics:
                            ctx_t[(hp, ic)] = ctxp.tile(
                                [HD + 1, CHUNK], f32, tag=f"ctx{hp}{ic % 2}",
                                name=f"ctx_{p}_{hp}_{ic}_{icp}")
                    pending = []  # deferred (ctx-mm emit, is_stop, ic)

                    def flush_ctx():
                        fn, is_stop, ic = pending.pop(0)
                        fn()
                        if is_stop:
                            # reciprocal of the rowsum rows: queue on DVE now
                            # (runs during later exps), broadcast+store later
                            for hp in range(2):
                                ct = ctx_t[(hp, ic)]
                                rsb = rsp.tile([1, CHUNK], bf16, tag="rsb")
                                with nc.allow_low_precision(
                                        reason="rowsum reciprocal to bf16"):
                                    nc.vector.reciprocal(rsb[:],
                                                         ct[HD:HD + 1, :])
                                post.append(
                                    lambda hp=hp, ic=ic, ct=ct, rsb=rsb:
                                    normalize_phase2(p, scp, rsp, hp, ic, ct,
                                                     rsb))

                    for jt in range(8 * icp + 8):
                        for ic in ics:
                            if 128 * jt >= CHUNK * (ic + 1):
                                continue
                            off = max(0, 128 * jt - CHUNK * ic)
                            # one side/post item per step: PE work that
                            # runs during this step's exp
                            if post:
                                post.pop(0)()
                            elif side:
                                side.pop(0)()
                            sc = scp.tile([128, 2, CHUNK], f32, tag="sc")
                            for hp in range(2):
                                _mm(nc.tensor.matmul(
                                    sc[:, hp, off:CHUNK],
                                    kt_sb[hp * 64:hp * 64 + 64, p,
                                          jt * 128:(jt + 1) * 128],
                                    qt_sb[hp * 64:hp * 64 + 64, p,
                                          ic * CHUNK + off:(ic + 1) * CHUNK],
                                    start=True,
                                    stop=True,
                                ))
                            pt = ptp.tile([128, 2, CHUNK], bf16, tag="pt")
                            nc.scalar.activation(
                                pt[:, :, off:CHUNK],
                                sc[:, :, off:CHUNK],
                                EXP,
                                scale=float(SCALE),
                            )
                            if jt // 4 == ic:  # diagonal block
                                nc.gpsimd.tensor_tensor(
                                    out=pt[:, :, off:off + 128],
                                    in0=pt[:, :, off:off + 128],
                                    in1=mask2_sb[:],
                                    op=MUL,
                                )

                            def emit_ctx(jt=jt, ic=ic, off=off, pt=pt):
                                for hp in range(2):
                                    h = 2 * p + hp
                                    _mm(nc.tensor.matmul(
                                        ctx_t[(hp, ic)][:, off:CHUNK],
                                        vj_sb[:, jt,
                                              h * (HD + 1):
                                              (h + 1) * (HD + 1)],
                                        pt[:, hp, off:CHUNK],
                                        start=(jt == 0),
                                        stop=(jt == 4 * ic + 3),
                                    ))
                            pending.append((emit_ctx, jt == 4 * ic + 3, ic))
                            if len(pending) > LAG:
                                flush_ctx()
                    while pending:
                        flush_ctx()

            # ---- phase structure -----------------------------------------
            with (
                tc.tile_pool(name="sc", bufs=2, space="PSUM") as scp,
                tc.tile_pool(name="ctx", bufs=1, space="PSUM") as ctxp,
                tc.tile_pool(name="rs", bufs=6) as rsp,
                tc.tile_pool(name="pt", bufs=LAG + 2) as ptp,
                tc.tile_pool(name="osb", bufs=3) as osbp,
            ):
                def emit_outproj(it):
                    ob = scp.tile([128, 2, CHUNK], f32, tag="sc",
                                  name=f"ob_{it}")
                    for p in range(2):
                        for nck in range(2):
                            _mm(nc.tensor.matmul(
                                ob[:, nck, :],
                                ctxT2_sb[:, p, it * 128:(it + 1) * 128],
                                wo_sb[:, p, nck * CHUNK:(nck + 1) * CHUNK],
                                start=(p == 0),
                                stop=(p == 1),
                            ))
                    osb = osbp.tile([128, 2, CHUNK], bf16, tag="osb")
                    nc.vector.tensor_copy(osb[:], ob[:])
                    nc.sync.dma_start(
                        outp.ap()[it * 128:(it + 1) * 128, :],
                        osb[:],
                    )

                kq = lambda n, w, d, pl, ic: (lambda: emit_kq_group(
                    scp, n, w, d, pl, ic))
                vg = lambda jt: (lambda: emit_v_group(scp, jt))

                # upfront: exactly what the first attention steps need
                emit_kq_group(scp, "k", wk_sb, kt_sb, 0, 0)
                emit_kq_group(scp, "q", wq_sb, qt_sb, 0, 0)
                emit_kq_group(scp, "q", wq_sb, qt_sb, 0, 1)
                emit_v_group(scp, 0)
                emit_v_group(scp, 1)

                # side work for pair 0, ordered by when results are needed
                side = [
                    kq("k", wk_sb, kt_sb, 0, 1), vg(2), vg(3),
                    kq("q", wq_sb, qt_sb, 0, 2), vg(4),
                    kq("k", wk_sb, kt_sb, 0, 2), vg(5),
                    kq("q", wq_sb, qt_sb, 0, 3), vg(6),
                    kq("k", wk_sb, kt_sb, 0, 3), vg(7), vg(8),
                    kq("k", wk_sb, kt_sb, 1, 0), vg(9),
                    kq("q", wq_sb, qt_sb, 1, 0), vg(10),
                    kq("q", wq_sb, qt_sb, 1, 1), vg(11),
                    kq("k", wk_sb, kt_sb, 1, 1), vg(12), vg(13),
                    kq("k", wk_sb, kt_sb, 1, 2), vg(14),
                    kq("q", wq_sb, qt_sb, 1, 2), vg(15),
                    kq("k", wk_sb, kt_sb, 1, 3),
                    kq("q", wq_sb, qt_sb, 1, 3),
                ]
                post = []
                attention_pair(0, scp, ctxp, rsp, ptp, side, post)
                attention_pair(1, scp, ctxp, rsp, ptp, side, post)
                while post:
                    post.pop(0)()
                for fn in side:
                    fn()
                for it in range(NIT):
                    emit_outproj(it)

    _split_multiwaits(nc)
    return nc


_nc_cache = None


def _get_program():
    global _nc_cache
    if _nc_cache is None:
        _nc_cache = _build_program()
    return _nc_cache


def kernel(hidden_states, Wq, bq, Wk, bk, Wv, bv, Wo, bo):
    from concourse.bass_utils import run_bass_kernel_spmd

    hidden_states = np.asarray(hidden_states, dtype=np.float32)
    Wq, bq = np.asarray(Wq, np.float32), np.asarray(bq, np.float32)
    Wk, bk = np.asarray(Wk, np.float32), np.asarray(bk, np.float32)
    Wv, bv = np.asarray(Wv, np.float32), np.asarray(bv, np.float32)
    Wo, bo = np.asarray(Wo, np.float32), np.asarray(bo, np.float32)

    ones64 = np.ones((1, 64), ml_dtypes.bfloat16)
    # mask[j_local, plane, i_local] = 1 where i >= j inside a diagonal block
    m = np.tril(np.ones((128, 128), np.float32)).T
    mask2 = np.ascontiguousarray(
        np.stack([m, m], axis=1).reshape(128, 256)).astype(ml_dtypes.bfloat16)
    onesvj = np.ones((128, NIT), ml_dtypes.bfloat16)

    def ktile_major(a, kt):  # [kt*128, F] -> [128, kt*F] partition-contig
        f = a.shape[1]
        return np.ascontiguousarray(
            a.reshape(kt, 128, f).transpose(1, 0, 2).reshape(128, kt * f)
        ).astype(ml_dtypes.bfloat16)

    in_maps = []
    for c in range(NCORES):
        b = c // (NCORES // B)
        hg = c % (NCORES // B)
        hsel = slice(hg * HPC * HD, (hg + 1) * HPC * HD)
        xt_n = ktile_major(np.ascontiguousarray(hidden_states[b].T), KT)
        bq_c = bq[hsel].reshape(2, 128).T
        bk_c = bk[hsel].reshape(2, 128).T
        bqkv_c = np.concatenate([bq_c, bk_c], axis=1).astype(np.float32)
        in_maps.append({
            "xt": xt_n,
            "wq": ktile_major(np.ascontiguousarray(Wq[:, hsel]), KT),
            "wk": ktile_major(np.ascontiguousarray(Wk[:, hsel]), KT),
            "wv": ktile_major(np.ascontiguousarray(Wv[:, hsel]), KT),
            "wo": ktile_major(np.ascontiguousarray(Wo[hsel, :]), 2),
            "bqkv": np.ascontiguousarray(bqkv_c),
            "ones64": ones64,
            "mask2": mask2,
            "onesvj": onesvj,
        })

    res = run_bass_kernel_spmd(_get_program(), in_maps, list(range(NCORES)))
    out = np.zeros((B, S, H), np.float32)
    for c in range(NCORES):
        out[c // (NCORES // B)] += np.asarray(
            res.results[c]["outp"]).astype(np.float32)
    out += (bo + bv @ Wo)[None, None, :]
    return out
